# revision 2
# baseline (speedup 1.0000x reference)
"""MoE all-to-all dispatch + combine (nn_EpAll2AllFusedOp) on 8 trn2 NeuronCores.

Semantics (matching the jax reference):
  flat_expert = topk_idx.reshape(T*K)
  sort_idx    = stable argsort of flat_expert
  dispatched  = x[sort_idx // K]                      # [T*K, H], expert-contiguous
  combined[t] = x[t] * sum_k topk_weights[t, k]      # gather-back of the K copies
  tokens_per_expert = histogram(flat_expert, 64)     # int32

Sharding: the dispatched buffer (T*K = 32768 rows, the 512 MB payload) is
split into 8 equal contiguous row slices - one per core (this is the
expert-parallel split of the sorted/A2A'd buffer, load-balanced by slot
rather than by raw expert id). Each core indirect-DMA-gathers its 4096 rows
of x (replicated input) and writes its dispatched slice; it also computes
its 512-token slice of `combined` (read x rows, scale by per-token weight
sum, write). The sort itself is O(T*K) integer metadata computed on host;
all tensor traffic (~144 MB/core) runs on-device.
"""

import numpy as np

import concourse.bass as bass
import concourse.mybir as mybir
import concourse.tile as tile
from concourse import bacc
from concourse.bass_utils import run_bass_kernel_spmd

T = 4096          # tokens
H = 4096          # hidden
K = 8             # topk
E = 64            # experts
NCORES = 8
RPC = T * K // NCORES       # dispatched rows per core = 4096
NTILES = RPC // 128         # gather tiles per core = 32
CTOK = T // NCORES          # combine tokens per core = 512
CTILES = CTOK // 128        # combine tiles per core = 4

FP32 = mybir.dt.float32
I32 = mybir.dt.int32


def build_module(n_reps: int = 1, gather_bufs: int = 6, comb_bufs: int = 3):
    """Build the SPMD Bass program (identical on all 8 cores).

    n_reps > 1 repeats the whole body (idempotent writes) - used only by the
    timing harness to amortize dispatch overhead out of wall-clock deltas.
    """
    nc = bacc.Bacc("TRN2", num_devices=NCORES)
    x = nc.dram_tensor("x", [T, H], FP32, kind="ExternalInput")
    idx = nc.dram_tensor("idx", [128, NTILES], I32, kind="ExternalInput")
    w = nc.dram_tensor("w", [128, CTILES * K], FP32, kind="ExternalInput")
    xc = nc.dram_tensor("xc", [CTOK, H], FP32, kind="ExternalInput")
    disp = nc.dram_tensor("disp", [RPC, H], FP32, kind="ExternalOutput")
    comb = nc.dram_tensor("comb", [CTOK, H], FP32, kind="ExternalOutput")

    with tile.TileContext(nc) as tc:
        with (
            tc.tile_pool(name="meta", bufs=1) as meta,
            tc.tile_pool(name="gpool", bufs=gather_bufs) as gpool,
            tc.tile_pool(name="cpool", bufs=comb_bufs) as cpool,
        ):
            idx_sb = meta.tile([128, NTILES], I32)
            nc.sync.dma_start(out=idx_sb[:], in_=idx[:])
            w_sb = meta.tile([128, CTILES * K], FP32)
            nc.sync.dma_start(out=w_sb[:], in_=w[:])
            ws = [meta.tile([128, 1], FP32, name=f"ws{j}", tag=f"ws{j}")
                  for j in range(CTILES)]
            for j in range(CTILES):
                nc.vector.reduce_sum(
                    out=ws[j][:], in_=w_sb[:, j * K:(j + 1) * K],
                    axis=mybir.AxisListType.X,
                )
            for _ in range(n_reps):
                for i in range(NTILES):
                    g = gpool.tile([128, H], FP32, tag="g")
                    nc.gpsimd.indirect_dma_start(
                        out=g[:],
                        out_offset=None,
                        in_=x[:],
                        in_offset=bass.IndirectOffsetOnAxis(
                            ap=idx_sb[:, i:i + 1], axis=0),
                    )
                    nc.sync.dma_start(out=disp[i * 128:(i + 1) * 128, :], in_=g[:])
                for j in range(CTILES):
                    xt = cpool.tile([128, H], FP32, tag="c")
                    nc.scalar.dma_start(
                        out=xt[:], in_=xc[j * 128:(j + 1) * 128, :])
                    nc.vector.tensor_scalar_mul(
                        out=xt[:], in0=xt[:], scalar1=ws[j][:])
                    nc.sync.dma_start(
                        out=comb[j * 128:(j + 1) * 128, :], in_=xt[:])
    nc.compile()
    return nc


def make_in_maps(x, topk_idx, topk_weights):
    """Host-side routing metadata + per-core input maps."""
    x = np.ascontiguousarray(x, dtype=np.float32)
    flat = np.ascontiguousarray(topk_idx, dtype=np.int32).reshape(-1)
    w_full = np.ascontiguousarray(topk_weights, dtype=np.float32)

    sort_idx = np.argsort(flat, kind="stable")
    src_tok = (sort_idx // K).astype(np.int32)
    tokens_per_expert = np.bincount(flat, minlength=E).astype(topk_idx.dtype)

    in_maps = []
    for c in range(NCORES):
        sl = src_tok[c * RPC:(c + 1) * RPC]
        idx_arr = np.ascontiguousarray(sl.reshape(NTILES, 128).T)
        wc = w_full[c * CTOK:(c + 1) * CTOK]
        w_arr = np.ascontiguousarray(
            wc.reshape(CTILES, 128, K).transpose(1, 0, 2).reshape(128, CTILES * K))
        in_maps.append({
            "x": x,
            "idx": idx_arr,
            "w": w_arr,
            "xc": np.ascontiguousarray(x[c * CTOK:(c + 1) * CTOK]),
        })
    return in_maps, tokens_per_expert


_module_cache = {}


def kernel(x, topk_idx, topk_weights):
    key = "main"
    if key not in _module_cache:
        _module_cache[key] = build_module()
    nc = _module_cache[key]

    in_maps, tokens_per_expert = make_in_maps(x, topk_idx, topk_weights)
    res = run_bass_kernel_spmd(nc, in_maps, core_ids=list(range(NCORES)))
    dispatched = np.concatenate([r["disp"] for r in res.results], axis=0)
    combined = np.concatenate([r["comb"] for r in res.results], axis=0)
    return combined, dispatched, tokens_per_expert
